# revision 1
# baseline (speedup 1.0000x reference)
"""BigBird block Trainium2 kernel: 8-core SPMD.

Sharding: core c -> batch b = c//4, group rank g = c%4.
  - attention: heads 4g..4g+3 (C=256 qkv cols), tensor-parallel
  - Wo partials ReduceScattered over token chunks: rank g receives the
    group-summed attention output for tokens [512g, 512(g+1))
  - FFN: token-parallel with the FULL 4096 hidden dim per core; each
    core emits the final x2+ff for its own 512 tokens (no second
    collective, no host-side partial summing)

All compute uses feature-major ("transposed") layouts [feature, token] so
matmul contractions keep features on partitions.  LN1 is folded into the
QKV projections (per-token mu/rsig applied post-matmul); LN2 is explicit.
Softmax denominators ride along the attn@V matmul via a ones column
appended to each V tile (65-wide per-head stationary operand).
q/k/attn-out/Wo and the mask run in bf16; the x/QKV and FFN matmuls run
as error-compensated fp8 DoubleRow (operand = fp8 main + fp8 residual,
three first-order product terms accumulated in f32 PSUM), which is 0.75x
the bf16 PE time at near-bf16 accuracy.
"""
import sys
from contextlib import ExitStack

sys.path.insert(0, "/opt/trn_rl_repo")
import numpy as np
import concourse.bacc as bacc
import concourse.mybir as mybir
from concourse import tile

F32 = mybir.dt.float32
F32R = mybir.dt.float32r
BF16 = mybir.dt.bfloat16
FP8 = mybir.dt.float8e4
NPBF16 = mybir.dt.np(BF16)
NPFP8 = mybir.dt.np(FP8)
W8SCALE = 64.0   # fp8 weight pre-scale (keeps 0.02-scale weights normal)

B, T, D, H, HD = 2, 2048, 1024, 16, 64
C = 256          # qkv cols per core (4 heads)
FF = 4096        # full ffn hidden (token-parallel ffn)
TC = 512         # tokens per core after reduce-scatter
NCORES = 8
GROUPS = [[0, 1, 2, 3], [4, 5, 6, 7]]
DT8 = D // 128   # 8 d-tiles
TT16 = T // 128  # 16 token tiles
HM32 = FF // 128  # 32 hidden tiles
LN_EPS = 1e-5

AF = mybir.ActivationFunctionType
OP = mybir.AluOpType


def r32(ap):
    return ap.bitcast(F32R)


def osl_o(om):
    return slice(om * 128, (om + 1) * 128)


def build_nc():
    nc = bacc.Bacc("TRN2", target_bir_lowering=False, debug=False,
                   num_devices=NCORES)
    dt = nc.dram_tensor
    # x and qkv weights ship as fp8 main+residual DoubleRow pair layouts
    # [dpair, 128, 2, cols]; weights pre-scaled by W8SCALE (folded back out
    # through the -rsig/W8SCALE constant in the LN1 epilogue)
    x8T = dt("x8T", [4, 128, 2, T], FP8, kind="ExternalInput")
    xdT = dt("xdT", [4, 128, 2, T], FP8, kind="ExternalInput")
    xTc = dt("xTc", [D, TC], F32, kind="ExternalInput")
    maskT = dt("maskT", [T, T], BF16, kind="ExternalInput")
    wqkv8 = dt("wqkv8", [4, 128, 2, 3 * C], FP8, kind="ExternalInput")
    wqkvd = dt("wqkvd", [4, 128, 2, 3 * C], FP8, kind="ExternalInput")
    wo = dt("wo", [C, D], BF16, kind="ExternalInput")
    # fp8 DoubleRow W1 with fp8 residual: [hg, 128, 2, dp*512+h] layout,
    # values pre-scaled by W8SCALE; w1d is the quantization remainder
    w18 = dt("w18", [8, 128, 2, 2048], FP8, kind="ExternalInput")
    w1d = dt("w1d", [8, 128, 2, 2048], FP8, kind="ExternalInput")
    w28 = dt("w28", [4, 128, 2, 4096], FP8, kind="ExternalInput")
    w2d = dt("w2d", [4, 128, 2, 4096], FP8, kind="ExternalInput")
    # scal blob: wsq|wsk|bq|bk (2 cols each) then wsv_bc|bv_bc (C each)
    scal_b = dt("scal_b", [128, 8 + 2 * C], F32, kind="ExternalInput")
    # col blob: bo (DT8) | b1 (HM32) | ws1 (HM32)
    col_b = dt("col_b", [128, DT8 + 2 * HM32], F32, kind="ExternalInput")

    xout = dt("xoutT", [D, TC], F32, kind="ExternalOutput")
    ar_in = dt("ar_in", [4, D, TC], BF16, kind="Internal")
    ar_out = dt("ar_out", [D, TC], BF16, kind="Internal")

    with ExitStack() as es:
        es.enter_context(nc.allow_low_precision(
            reason="bf16/fp32r SBUF tiles feed the PE; accumulation stays f32"))
        tc = es.enter_context(tile.TileContext(nc))

        def pool(name, bufs, space="SBUF"):
            return tc.tile_pool(name=name, bufs=bufs, space=space)

        pp = es.enter_context(pool("persist", 1))
        ones_sb = pp.tile([128, 128], F32R, name="ones_sb")
        nc.gpsimd.memset(ones_sb[:].bitcast(F32), 1.0)
        ones_sbf = pp.tile([128, 128], BF16, name="ones_sbf")
        nc.gpsimd.memset(ones_sbf[:], 1.0)
        inv128 = pp.tile([128, 1], F32R, name="inv128")
        nc.gpsimd.memset(inv128[:].bitcast(F32), 1.0 / 128.0)
        ones_bf = pp.tile([1, 128], BF16, name="ones_bf")
        nc.gpsimd.memset(ones_bf[:], 1.0)
        invw8 = pp.tile([128, 1], F32, name="invw8")
        nc.gpsimd.memset(invw8[:], 1.0 / W8SCALE)
        ones_p8 = pp.tile([128, 128], FP8, name="ones_p8")
        nc.gpsimd.memset(ones_p8[:], 1.0)
        epsc = pp.tile([128, 1], F32, name="epsc")
        nc.gpsimd.memset(epsc[:], LN_EPS)

        # qT/kT/v live from phase 2 through phase 4
        s234 = es.enter_context(ExitStack())
        qsb = s234.enter_context(pool("qkv_sb", 1))

        # ===== phases 1-4: LN1 + QKV + attention, chunk-pipelined ========
        # x tiles first (LN1 stats are the very first PE work); the qkv
        # weights are not needed until the v/QKV matmuls ~15us later.
        xres = s234.enter_context(pool("xres", 1))
        x8ts, xdts = [], []
        for dp in range(4):
            t_ = xres.tile([128, 2, T], FP8, tag=f"x8_{dp}", name=f"x8_{dp}")
            nc.sync.dma_start(t_[:], x8T[dp])
            x8ts.append(t_)
        for dp in range(4):
            t_ = xres.tile([128, 2, T], FP8, tag=f"xd_{dp}", name=f"xd_{dp}")
            nc.sync.dma_start(t_[:], xdT[dp])
            xdts.append(t_)

        def x8s(d):
            return x8ts[d // 2][:, d % 2, :]

        DR8 = mybir.MatmulPerfMode.DoubleRow

        wp = s234.enter_context(pool("wqkv", 1))
        w8_sb, wd_sb = [], []
        for dp in range(4):
            w_ = wp.tile([128, 2, 3 * C], FP8, tag=f"w8_{dp}",
                         name=f"w8_{dp}")
            nc.sync.dma_start(w_[:], wqkv8[dp])
            w8_sb.append(w_)
        for dp in range(4):
            w_ = wp.tile([128, 2, 3 * C], FP8, tag=f"wd_{dp}",
                         name=f"wd_{dp}")
            nc.sync.dma_start(w_[:], wqkvd[dp])
            wd_sb.append(w_)
        scalt = wp.tile([128, 8 + 2 * C], F32, tag="scal", name="scal")
        nc.sync.dma_start(scalt[:], scal_b[:])
        scal = {"wsq": scalt[:, 0:2], "wsk": scalt[:, 2:4],
                "bq": scalt[:, 4:6], "bk": scalt[:, 6:8]}
        wsv_sb = scalt[:, 8:8 + C]
        bv_sb = scalt[:, 8 + C:8 + 2 * C]

        statsb = s234.enter_context(pool("statsb", 1))
        mu_bc = statsb.tile([128, T], F32, tag="mu", name="mu")
        nrsig_bc = statsb.tile([128, T], F32, tag="nrsig", name="nrsig")
        murs_bc = statsb.tile([128, T], F32, tag="murs", name="murs")
        wrk = statsb.tile([128, T], F32, tag="wrk", name="wrk")

        # LN1 stats, chunked by 512-token groups so downstream work can
        # start as soon as the first chunk's mu/rsig are ready.
        rsig_col, murs_col = [None] * TT16, [None] * TT16
        with pool("sqp", 3) as sqp, pool("statps", 1, "PSUM") as statps:
            sum_ps = [statps.tile([128, 512], F32, tag=f"sum{n}", name=f"sum{n}")
                      for n in range(4)]
            sq_ps = [statps.tile([128, 512], F32, tag=f"sq{n}", name=f"sq{n}")
                     for n in range(4)]
            for d in range(DT8):
                sq = sqp.tile([128, T], BF16, tag="sq", name="sq")
                nc.gpsimd.tensor_tensor(sq[:], x8s(d), x8s(d), OP.mult)
                for n in range(4):
                    sl = slice(n * 512, (n + 1) * 512)
                    nc.tensor.matmul(sum_ps[n][:], ones_p8[:],
                                     x8s(d)[:, sl],
                                     start=(d == 0), stop=(d == DT8 - 1),
                                     skip_group_check=True)
                    nc.tensor.matmul(sq_ps[n][:], ones_sbf[:],
                                     sq[:, sl],
                                     start=(d == 0), stop=(d == DT8 - 1),
                                     skip_group_check=True)
            for n in range(4):
                sl = slice(n * 512, (n + 1) * 512)
                nc.scalar.activation(mu_bc[:, sl], sum_ps[n][:], AF.Copy,
                                     scale=1.0 / D)
                nc.scalar.activation(wrk[:, sl], sq_ps[n][:], AF.Copy,
                                     scale=1.0 / D)
                # var = E[x^2] - mu^2 + eps; rsig = 1/sqrt(var)
                nc.vector.tensor_tensor(murs_bc[:, sl], mu_bc[:, sl],
                                        mu_bc[:, sl], OP.mult)
                nc.vector.tensor_sub(wrk[:, sl], wrk[:, sl],
                                     murs_bc[:, sl])
                nc.scalar.activation(wrk[:, sl], wrk[:, sl], AF.Sqrt,
                                     bias=epsc[:])
                nc.vector.reciprocal(murs_bc[:, sl], wrk[:, sl])  # rsig
                # -rsig/W8SCALE: also unscales the x64 fp8 weight products
                nc.scalar.activation(nrsig_bc[:, sl], murs_bc[:, sl],
                                     AF.Copy, scale=-1.0 / W8SCALE)
                nc.vector.tensor_tensor(murs_bc[:, sl], mu_bc[:, sl],
                                        murs_bc[:, sl], OP.mult)
                # per-token scalar columns (for the v path); reuse the
                # consumed stats PSUM banks for the 1-col matmuls
                for ti, tt in enumerate(range(4 * n, 4 * n + 4)):
                    tsl = slice(tt * 128, (tt + 1) * 128)
                    pr = sum_ps[n][:, ti:ti + 1]
                    nc.tensor.matmul(pr, nrsig_bc[:, tsl],
                                     inv128[:].bitcast(F32),
                                     start=True, stop=True,
                                     skip_group_check=True)
                    rc = statsb.tile([128, 1], F32, tag=f"rc{tt}",
                                     name=f"rc{tt}")
                    nc.vector.tensor_scalar_mul(rc[:], pr, -1.0)
                    rsig_col[tt] = rc
                    pm = sq_ps[n][:, ti:ti + 1]
                    nc.tensor.matmul(pm, murs_bc[:, tsl],
                                     inv128[:].bitcast(F32),
                                     start=True, stop=True,
                                     skip_group_check=True)
                    mc = statsb.tile([128, 1], F32, tag=f"mc{tt}",
                                     name=f"mc{tt}")
                    nc.vector.tensor_scalar_mul(mc[:], pm, 1.0 / W8SCALE)
                    murs_col[tt] = mc

        # ---- v tiles upfront (natural [t-part, c-free] bf16 + ones col) --
        qtmp = s234.enter_context(pool("qtmp", 2))
        v_sb = []
        with pool("vps", 6, "PSUM") as vps:
            for tt in range(TT16):
                vt = qsb.tile([128, 4 * 65], BF16, tag=f"v{tt}", name=f"v{tt}")
                v3 = vt[:].rearrange("p (h c) -> p h c", h=4)
                nc.gpsimd.memset(v3[:, :, 64:65], 1.0)
                vp = vps.tile([128, C], F32, tag="vp", name="vp")
                tsl = slice(tt * 128, (tt + 1) * 128)
                vsl = slice(2 * C, 3 * C)
                vterms = ([(x8ts[dp], w8_sb[dp]) for dp in range(4)]
                          + [(xdts[dp], w8_sb[dp]) for dp in range(4)]
                          + [(x8ts[dp], wd_sb[dp]) for dp in range(4)])
                for i, (xt_, wt_) in enumerate(vterms):
                    nc.tensor.matmul(vp[:], xt_[:, :, tsl], wt_[:, :, vsl],
                                     start=(i == 0),
                                     stop=(i == len(vterms) - 1),
                                     perf_mode=DR8,
                                     skip_group_check=True)
                tmp2 = qtmp.tile([128, C], F32, tag="tmp2", name="tmp2")
                veng = nc.gpsimd if tt % 2 else nc.vector
                veng.tensor_scalar(tmp2[:], wsv_sb[:],
                                   murs_col[tt][:], None, OP.mult)
                veng.tensor_sub(tmp2[:], bv_sb[:], tmp2[:])
                vp3 = vp[:].rearrange("p (h c) -> p h c", h=4)
                t23 = tmp2[:].rearrange("p (h c) -> p h c", h=4)
                nc.vector.scalar_tensor_tensor(
                    v3[:, :, 0:64], vp3[:, :, :], rsig_col[tt][:],
                    t23[:, :, :], OP.mult, OP.add)
                v_sb.append(vt)

        # ---- attention persistent tiles ----
        asb = s234.enter_context(pool("att_sb", 1))
        attnT = [asb.tile([128, T], BF16, tag=f"aT{m}", name=f"aT{m}")
                 for m in range(2)]
        wo_sb = []
        for cc in range(2):
            w_ = asb.tile([128, D], BF16, tag=f"wo{cc}", name=f"wo{cc}")
            nc.sync.dma_start(w_[:], wo[cc * 128:(cc + 1) * 128, :])
            wo_sb.append(w_)
        qT = [[None] * 4 for _ in range(2)]
        kT = [[None] * 4 for _ in range(2)]

        with pool("mskp", 2) as mskp, pool("ptp", 6) as ptp, \
             pool("qkps", 1, "PSUM") as qkps, \
             pool("sps", 2, "PSUM") as spsp, \
             pool("avps", 1, "PSUM") as avps, \
             pool("dnps", 1, "PSUM") as dnps, \
             pool("dnb", 2) as dnb, pool("arp", 2) as arp:

            def qkv_group(n, zname, zoff, tab, ws_key, b_key, m):
                nsl = slice(n * 512, (n + 1) * 512)
                zt = qsb.tile([128, 512], BF16,
                              tag=f"{zname}T{m}n{n}",
                              name=f"{zname}T{m}n{n}")
                msl = slice(zoff + m * 128, zoff + (m + 1) * 128)
                zp = qkps.tile([128, 512], F32, tag="zp", name="zp")
                zterms = ([(w8_sb[dp], x8ts[dp]) for dp in range(4)]
                          + [(w8_sb[dp], xdts[dp]) for dp in range(4)]
                          + [(wd_sb[dp], x8ts[dp]) for dp in range(4)])
                for i, (wt_, xt_) in enumerate(zterms):
                    nc.tensor.matmul(
                        zp[:], wt_[:, :, msl], xt_[:, :, nsl],
                        start=(i == 0), stop=(i == len(zterms) - 1),
                        perf_mode=DR8, skip_group_check=True)
                # (mu*wsz - raw); then z = that*(-rsig) + b
                # SBUF-only follow-ups alternate onto gpsimd
                tmpz = qtmp.tile([128, 512], F32, tag="tmpz",
                                 name="tmpz")
                nc.vector.scalar_tensor_tensor(
                    tmpz[:], mu_bc[:, nsl],
                    scal[ws_key][:, m:m + 1], zp[:],
                    OP.mult, OP.subtract)
                zeng = nc.gpsimd if n % 2 else nc.vector
                zeng.tensor_tensor(tmpz[:], tmpz[:],
                                   nrsig_bc[:, nsl], OP.mult)
                zeng.tensor_scalar(zt[:], tmpz[:],
                                   scal[b_key][:, m:m + 1],
                                   None, OP.add)
                tab[m][n] = zt

            def qkv_fillers(n):
                return [
                    (lambda zn=zn, zo=zo, tab=tab, wk_=wk_, bk_=bk_, m=m:
                     qkv_group(n, zn, zo, tab, wk_, bk_, m))
                    for zn, zo, tab, wk_, bk_ in (
                        ("q", 0, qT, "wsq", "bq"),
                        ("k", C, kT, "wsk", "bk"))
                    for m in range(2)]

            def attn_chunk(j, fillers=()):
                fillers = list(fillers)
                n_kt = 4 * j + 4
                qsl = slice(j * 512, (j + 1) * 512)
                # one batched mask DMA per chunk: [p, kt, q] <- maskT rows
                # (single rotating tag keeps far-future chunks from
                # hoisting their loads ahead of the critical x tiles)
                mj = mskp.tile([128, n_kt, 512], BF16, tag="mj",
                               name=f"mj{j}")
                nc.sync.dma_start(
                    mj[:], maskT[0:n_kt * 128, qsl]
                    .rearrange("(kt p) q -> p kt q", p=128))
                mts = [mj[:, kt, :] for kt in range(n_kt)]
                den = [dnb.tile([1, 512], F32, tag=f"den{h}",
                                name=f"den{h}j{j}") for h in range(4)]
                rden = [dnb.tile([1, 512], BF16, tag=f"rden{h}",
                                 name=f"rden{h}j{j}") for h in range(4)]
                for hp in range(2):
                    avA = avps.tile([65, 512], F32, tag="avA", name="avA")
                    avB = avps.tile([65, 512], F32, tag="avB", name="avB")
                    for kt in range(n_kt):
                        kts = kT[hp][kt // 4][:, (kt % 4) * 128:
                                              (kt % 4) * 128 + 128]
                        qts = qT[hp][j]
                        sps = spsp.tile([128, 1024], F32, tag="sps",
                                        name="sps")
                        nc.tensor.matmul(
                            sps[:, 0:512], kts[0:64, :],
                            qts[0:64, :], start=True, stop=True,
                            tile_position=(0, 0), skip_group_check=True)
                        nc.tensor.matmul(
                            sps[:, 512:1024], kts[64:128, :],
                            qts[64:128, :], start=True, stop=True,
                            tile_position=(64, 0), skip_group_check=True)
                        pt = ptp.tile([128, 1024], BF16, tag="pt", name="pt")
                        nc.scalar.activation(pt[:], sps[:], AF.Exp,
                                             scale=0.125)
                        # mask multiply: offload 1/3 to the idle gpsimd
                        meng = nc.gpsimd if kt % 3 == 2 else nc.vector
                        meng.tensor_mul(pt[:, 0:512], pt[:, 0:512],
                                        mts[kt][:])
                        meng.tensor_mul(pt[:, 512:1024],
                                        pt[:, 512:1024], mts[kt][:])
                        vv = v_sb[kt][:].rearrange("p (h c) -> p h c", h=4)
                        nc.tensor.matmul(
                            avA[:], vv[:, 2 * hp, :], pt[:, 0:512],
                            start=(kt == 0), stop=(kt == n_kt - 1),
                            skip_group_check=True)
                        nc.tensor.matmul(
                            avB[:], vv[:, 2 * hp + 1, :], pt[:, 512:1024],
                            start=(kt == 0), stop=(kt == n_kt - 1),
                            skip_group_check=True)
                        # independent PE filler (next chunk's QKV) to cover
                        # the exp->AV dependency stalls
                        idx = hp * n_kt + kt
                        if fillers and idx % max(1, (2 * n_kt) // 4) == 1:
                            fillers.pop(0)()
                    nc.vector.tensor_copy(attnT[hp][0:64, qsl],
                                          avA[0:64, :])
                    nc.vector.tensor_copy(attnT[hp][64:128, qsl],
                                          avB[0:64, :])
                    nc.vector.tensor_copy(den[2 * hp][:], avA[64:65, :])
                    nc.vector.tensor_copy(den[2 * hp + 1][:],
                                          avB[64:65, :])
                # normalize chunk j and project through Wo, then RS input j
                for h in range(4):
                    nc.vector.reciprocal(rden[h][:], den[h][:])
                for m in range(2):
                    dp = dnps.tile([128, 512], F32, tag="dn", name="dp")
                    nc.tensor.matmul(dp[0:64, :],
                                     ones_bf[0:1, 0:64],
                                     rden[2 * m][:],
                                     start=True, stop=True,
                                     skip_group_check=True)
                    nc.tensor.matmul(dp[64:128, :],
                                     ones_bf[0:1, 64:128],
                                     rden[2 * m + 1][:],
                                     start=True, stop=True,
                                     skip_group_check=True)
                    nc.vector.tensor_mul(attnT[m][:, qsl], attnT[m][:, qsl],
                                         dp[:])
                aoj = arp.tile([128, DT8, 512], BF16, tag="ao", name="ao")
                for o in range(DT8):
                    osl = slice(o * 128, (o + 1) * 128)
                    wps = dnps.tile([128, 512], F32, tag="dn", name="wps")
                    for cc in range(2):
                        nc.tensor.matmul(
                            wps[:], wo_sb[cc][:, osl],
                            attnT[cc][:, qsl],
                            start=(cc == 0), stop=(cc == 1),
                            skip_group_check=True)
                    if o % 2:
                        nc.scalar.copy(aoj[:, o, :], wps[:])
                    else:
                        nc.vector.tensor_copy(aoj[:, o, :], wps[:])
                nc.sync.dma_start(
                    ar_in[j].rearrange("(o p) q -> p o q", p=128),
                    aoj[:])
                for f in fillers:
                    f()

            for f in qkv_fillers(0):
                f()
            for n in range(4):
                attn_chunk(n, qkv_fillers(n + 1) if n < 3 else ())
            # one ReduceScatter over token chunks: rank g gets chunk g
            nc.gpsimd.collective_compute(
                "ReduceScatter", mybir.AluOpType.add,
                replica_groups=GROUPS,
                ins=[ar_in[:]], outs=[ar_out[:]])
        s234.close()  # free qT/kT/v/attnT SBUF before phases 5-6

        # ========= phases 5+6: own-chunk x2 + LN2 + full-hidden FFN ======
        with pool("x2p", 1) as x2p, pool("ln2sb", 1) as ln2sb, \
             pool("sqp2", 3) as sqp2, pool("arl", 1) as arl, \
             pool("p5ps", 1, "PSUM") as p5ps, \
             pool("w1p", 2) as w1p, pool("a1p", 1) as a1p, \
             pool("xop", 1) as xop, pool("colp", 1) as colp, \
             pool("w2p", 1) as w2p:
            colt = colp.tile([128, DT8 + 2 * HM32], F32, tag="colb",
                             name="colb")
            nc.sync.dma_start(colt[:], col_b[:])
            bo_sb = colt[:, 0:DT8]
            b1_sb = colt[:, DT8:DT8 + HM32]
            ws1_sb = colt[:, DT8 + HM32:DT8 + 2 * HM32]
            w28_sb, w2d_sb = [], []
            for g4 in range(4):
                wt = w2p.tile([128, 2, 4096], FP8, tag=f"w28_{g4}",
                              name=f"w28_{g4}")
                nc.sync.dma_start(wt[:], w28[g4])
                w28_sb.append(wt)
                wt = w2p.tile([128, 2, 4096], FP8, tag=f"w2d_{g4}",
                              name=f"w2d_{g4}")
                nc.sync.dma_start(wt[:], w2d[g4])
                w2d_sb.append(wt)

            # x2 = x + attn_out + bo for own tokens (batched loads)
            xr8 = arl.tile([128, DT8, TC], F32, tag="xr", name="xr")
            nc.sync.dma_start(
                xr8[:], xTc[:].rearrange("(d p) q -> p d q", p=128))
            ar8 = arl.tile([128, DT8, TC], BF16, tag="art", name="art")
            nc.sync.dma_start(
                ar8[:], ar_out[:].rearrange("(d p) q -> p d q", p=128))
            x2 = []
            for d in range(DT8):
                xt2 = x2p.tile([128, TC], F32R, tag=f"x2_{d}", name=f"x2_{d}")
                nc.vector.scalar_tensor_tensor(
                    xt2[:], ar8[:, d, :], bo_sb[:, d:d + 1], xr8[:, d, :],
                    OP.add, OP.add)
                x2.append(xt2)
            # LN2 stats for own chunk
            sum_ps = p5ps.tile([128, TC], F32, tag="s2", name="s2")
            sq_ps = p5ps.tile([128, TC], F32, tag="q2", name="q2")
            for d in range(DT8):
                sq = sqp2.tile([128, TC], F32R, tag="sq2", name="sq2")
                nc.scalar.activation(sq[:], x2[d][:], AF.Square)
                nc.tensor.matmul(sum_ps[:], r32(ones_sb[:]),
                                 r32(x2[d][:]),
                                 start=(d == 0), stop=(d == DT8 - 1),
                                 skip_group_check=True)
                nc.tensor.matmul(sq_ps[:], r32(ones_sb[:]), r32(sq[:]),
                                 start=(d == 0), stop=(d == DT8 - 1),
                                 skip_group_check=True)
            mu2 = ln2sb.tile([128, TC], F32, tag="mu2", name="mu2")
            nrsig2 = ln2sb.tile([128, TC], F32, tag="nrsig2", name="nrsig2")
            wrk2 = ln2sb.tile([128, TC], F32, tag="wrk2", name="wrk2")
            nc.scalar.activation(mu2[:], sum_ps[:], AF.Copy, scale=1.0 / D)
            nc.scalar.activation(wrk2[:], sq_ps[:], AF.Copy, scale=1.0 / D)
            nc.vector.tensor_tensor(nrsig2[:], mu2[:], mu2[:], OP.mult)
            nc.vector.tensor_sub(wrk2[:], wrk2[:], nrsig2[:])
            nc.scalar.activation(wrk2[:], wrk2[:], AF.Sqrt, bias=epsc[:])
            nc.vector.reciprocal(nrsig2[:], wrk2[:])
            murs2 = ln2sb.tile([128, TC], F32, tag="murs2", name="murs2")
            nc.vector.tensor_tensor(murs2[:], mu2[:], nrsig2[:], OP.mult)
            # x2s = x2 * rsig2, split into fp8 main + fp8 residual pair
            # tiles (DoubleRow operands); quantization error is first-order
            # compensated by the extra W8@dx and dW@x8 matmul terms
            x8p = [ln2sb.tile([128, 2, TC], FP8, tag=f"x8_{dp}",
                              name=f"x8_{dp}") for dp in range(DT8 // 2)]
            dxp = [ln2sb.tile([128, 2, TC], FP8, tag=f"dx_{dp}",
                              name=f"dx_{dp}") for dp in range(DT8 // 2)]
            for d in range(DT8):
                xeng = nc.gpsimd if d % 2 else nc.vector
                tf = sqp2.tile([128, TC], F32, tag="tf", name="tf")
                xeng.tensor_tensor(tf[:], x2[d][:], nrsig2[:], OP.mult)
                x8sl = x8p[d // 2][:, d % 2:d % 2 + 1, :]
                xeng.tensor_copy(x8sl, tf[:])
                xeng.tensor_sub(dxp[d // 2][:, d % 2:d % 2 + 1, :],
                                tf[:], x8sl)
            DR = mybir.MatmulPerfMode.DoubleRow
            # FFN1: stream w1 tiles by hidden-group; full 4096 hidden.
            # FFN2 for om 0..5 rides along (hm-pair outer, persistent PSUM
            # accumulators) so it fills PE stalls during FFN1. a1 is split
            # fp8 main+residual like x2s (gpsimd does the split).
            a18p = [a1p.tile([128, 2, TC], FP8, tag=f"a18_{hp}",
                             name=f"a18_{hp}") for hp in range(HM32 // 2)]
            da1p = [a1p.tile([128, 2, TC], FP8, tag=f"da1_{hp}",
                             name=f"da1_{hp}") for hp in range(HM32 // 2)]
            facc = [p5ps.tile([128, TC], F32, tag=f"facc{om}",
                              name=f"facc{om}") for om in range(4)]
            facc.append(p5ps.tile([128, TC], F32, tag="s2", name="facc4"))
            facc.append(p5ps.tile([128, TC], F32, tag="q2", name="facc5"))
            for hg in range(FF // 512):
                w8t = w1p.tile([128, 2, 2048], FP8, tag="w18g", name="w18g")
                nc.sync.dma_start(w8t[:], w18[hg])
                wdt = w1p.tile([128, 2, 2048], FP8, tag="w1dg", name="w1dg")
                nc.sync.dma_start(wdt[:], w1d[hg])
                for hl in range(4):
                    hm = hg * 4 + hl
                    ap_ = p5ps.tile([128, TC], F32, tag="a1ps",
                                    name="a1ps", bufs=2)
                    terms = ([(w8t, x8p[dp], dp) for dp in range(4)]
                             + [(w8t, dxp[dp], dp) for dp in range(4)]
                             + [(wdt, x8p[dp], dp) for dp in range(4)])
                    for i, (wt, xp, dp) in enumerate(terms):
                        hsl = slice(dp * 512 + hl * 128,
                                    dp * 512 + hl * 128 + 128)
                        nc.tensor.matmul(ap_[:], wt[:, :, hsl], xp[:, :, :],
                                         start=(i == 0),
                                         stop=(i == len(terms) - 1),
                                         perf_mode=DR,
                                         skip_group_check=True)
                    a1n = sqp2.tile([128, TC], F32, tag="a1n", name="a1n")
                    nc.vector.scalar_tensor_tensor(
                        a1n[:], murs2[:], ws1_sb[:, hm:hm + 1], ap_[:],
                        OP.mult, OP.subtract)
                    a1f = sqp2.tile([128, TC], BF16, tag="a1f", name="a1f")
                    nc.scalar.activation(a1f[:], a1n[:], AF.Gelu,
                                         bias=b1_sb[:, hm:hm + 1],
                                         scale=-1.0 / W8SCALE)
                    a18s = a18p[hm // 2][:, hm % 2:hm % 2 + 1, :]
                    nc.gpsimd.tensor_copy(a18s, a1f[:])
                    nc.gpsimd.tensor_sub(
                        da1p[hm // 2][:, hm % 2:hm % 2 + 1, :],
                        a1f[:], a18s)
                    if hm % 2 == 1:
                        hp = hm // 2
                        g4, l4 = hp // 4, hp % 4
                        t2 = ([(w28_sb[g4], a18p[hp]),
                               (w28_sb[g4], da1p[hp]),
                               (w2d_sb[g4], a18p[hp])])
                        for om in range(6):
                            osl = slice(l4 * 1024 + om * 128,
                                        l4 * 1024 + om * 128 + 128)
                            for ti, (wt, ap2) in enumerate(t2):
                                nc.tensor.matmul(
                                    facc[om][:], wt[:, :, osl], ap2[:, :, :],
                                    start=(hp == 0 and ti == 0),
                                    stop=(hp == HM32 // 2 - 1 and ti == 2),
                                    perf_mode=DR, skip_group_check=True)
            xoA = xop.tile([128, 6, TC], F32, tag="xoA", name="xoA")
            for om in range(6):
                nc.vector.scalar_tensor_tensor(
                    xoA[:, om, :], facc[om][:], invw8[:], x2[om][:],
                    OP.mult, OP.add)
            nc.sync.dma_start(
                xout[0:768, :].rearrange("(o p) q -> p o q", p=128), xoA[:])
            # FFN2 remainder (om 6..7) after all a1 are ready
            xoB = xop.tile([128, 2, TC], F32, tag="xoB", name="xoB")
            for om in range(6, DT8):
                fp_ = p5ps.tile([128, TC], F32, tag="a1ps",
                                name="a1ps", bufs=2)
                for hp in range(HM32 // 2):
                    g4, l4 = hp // 4, hp % 4
                    osl = slice(l4 * 1024 + om * 128,
                                l4 * 1024 + om * 128 + 128)
                    t2 = ([(w28_sb[g4], a18p[hp]),
                           (w28_sb[g4], da1p[hp]),
                           (w2d_sb[g4], a18p[hp])])
                    for ti, (wt, ap2) in enumerate(t2):
                        nc.tensor.matmul(
                            fp_[:], wt[:, :, osl], ap2[:, :, :],
                            start=(hp == 0 and ti == 0),
                            stop=(hp == HM32 // 2 - 1 and ti == 2),
                            perf_mode=mybir.MatmulPerfMode.DoubleRow,
                            skip_group_check=True)
                nc.vector.scalar_tensor_tensor(
                    xoB[:, om - 6, :], fp_[:], invw8[:], x2[om][:],
                    OP.mult, OP.add)
            nc.sync.dma_start(
                xout[768:1024, :].rearrange("(o p) q -> p o q", p=128),
                xoB[:])
    nc.compile()
    return nc


def host_prep(inputs):
    """Build per-core input maps from the full problem inputs."""
    x = np.asarray(inputs["x"], np.float32)
    mask = np.asarray(inputs["mask"])
    ln1_g = np.asarray(inputs["ln1_g"], np.float32)
    ln1_b = np.asarray(inputs["ln1_b"], np.float32)
    ln2_g = np.asarray(inputs["ln2_g"], np.float32)
    ln2_b = np.asarray(inputs["ln2_b"], np.float32)
    Wq = np.asarray(inputs["Wq"], np.float32)
    Wk = np.asarray(inputs["Wk"], np.float32)
    Wv = np.asarray(inputs["Wv"], np.float32)
    Wo = np.asarray(inputs["Wo"], np.float32)
    bo = np.asarray(inputs["bo"], np.float32)
    W1 = np.asarray(inputs["W1"], np.float32)
    b1 = np.asarray(inputs["b1"], np.float32)
    W2 = np.asarray(inputs["W2"], np.float32)
    b2 = np.asarray(inputs["b2"], np.float32)

    maskT = np.ascontiguousarray(mask.T).astype(np.float32).astype(NPBF16)
    Wq_f = ln1_g[:, None] * Wq
    Wk_f = ln1_g[:, None] * Wk
    Wv_f = ln1_g[:, None] * Wv
    W1_f = ln2_g[:, None] * W1
    w1_64 = W8SCALE * W1_f
    w18v = w1_64.astype(NPFP8)
    w1dv = (w1_64 - w18v.astype(np.float32)).astype(NPFP8)

    def dr_layout(w):  # [D, FF] -> [hg, p, i, dp*512+h]
        return np.ascontiguousarray(
            w.reshape(4, 2, 128, 8, 512).transpose(3, 2, 1, 0, 4)
            .reshape(8, 128, 2, 2048))

    w1_eff = w18v.astype(np.float32) + w1dv.astype(np.float32)  # 64x scale
    ws1 = w1_eff.sum(0)
    b1_full = ln2_b @ (w1_eff / W8SCALE) + b1
    w18_dr = dr_layout(w18v)
    w1d_dr = dr_layout(w1dv)
    w2_64 = W8SCALE * W2
    w28v = w2_64.astype(NPFP8)
    w2dv = (w2_64 - w28v.astype(np.float32)).astype(NPFP8)

    def dr2_layout(w):  # [FF, D] -> [g4, p, i, l4*1024+o]
        return np.ascontiguousarray(
            w.reshape(4, 4, 2, 128, D).transpose(0, 3, 2, 1, 4)
            .reshape(4, 128, 2, 4 * D))

    w28_dr = dr2_layout(w28v)
    w2d_dr = dr2_layout(w2dv)
    in_maps = []
    def drx_layout(w):  # [D, cols] -> [dpair, 128, 2, cols]
        return np.ascontiguousarray(
            w.reshape(4, 2, 128, w.shape[1]).transpose(0, 2, 1, 3))

    for c in range(NCORES):
        b, g = divmod(c, 4)
        cs = slice(g * C, (g + 1) * C)
        xTb = np.ascontiguousarray(x[b].T)
        x8v = xTb.astype(NPFP8)
        xdv = (xTb - x8v.astype(np.float32)).astype(NPFP8)
        wqkv_64 = W8SCALE * np.concatenate(
            [Wq_f[:, cs], Wk_f[:, cs], Wv_f[:, cs]], axis=1)
        wqkv8v = wqkv_64.astype(NPFP8)
        wqkvdv = (wqkv_64 - wqkv8v.astype(np.float32)).astype(NPFP8)
        wqkv_eff = wqkv8v.astype(np.float32) + wqkvdv.astype(np.float32)
        scal = np.empty((128, 8 + 2 * C), np.float32)
        scal[:, 0:2] = wqkv_eff[:, 0:C].sum(0).reshape(2, 128).T
        scal[:, 2:4] = wqkv_eff[:, C:2 * C].sum(0).reshape(2, 128).T
        scal[:, 4:6] = (ln1_b @ Wq[:, cs]).reshape(2, 128).T
        scal[:, 6:8] = (ln1_b @ Wk[:, cs]).reshape(2, 128).T
        scal[:, 8:8 + C] = wqkv_eff[:, 2 * C:3 * C].sum(0)[None, :]
        scal[:, 8 + C:8 + 2 * C] = (ln1_b @ Wv[:, cs])[None, :]
        colb = np.empty((128, DT8 + 2 * HM32), np.float32)
        colb[:, 0:DT8] = bo.reshape(DT8, 128).T
        colb[:, DT8:DT8 + HM32] = b1_full.reshape(HM32, 128).T
        colb[:, DT8 + HM32:] = ws1.reshape(HM32, 128).T
        m = {
            "x8T": drx_layout(x8v),
            "xdT": drx_layout(xdv),
            "xTc": np.ascontiguousarray(xTb[:, g * TC:(g + 1) * TC]),
            "maskT": maskT,
            "wqkv8": drx_layout(wqkv8v),
            "wqkvd": drx_layout(wqkvdv),
            "wo": np.ascontiguousarray(Wo[cs, :]).astype(NPBF16),
            "w18": w18_dr,
            "w1d": w1d_dr,
            "w28": w28_dr,
            "w2d": w2d_dr,
            "scal_b": scal,
            "col_b": colb,
        }
        in_maps.append(m)
    return in_maps, b2


def host_assemble(out_maps, b2):
    out = np.empty((B, T, D), np.float32)
    for c in range(NCORES):
        b, g = divmod(c, 4)
        out[b, g * TC:(g + 1) * TC, :] = out_maps[c]["xoutT"].T + b2
    return out


# ======================================================================
# Harness entry point
# ======================================================================
_NC_CACHE = {}


def _get_nc():
    if "nc" not in _NC_CACHE:
        _NC_CACHE["nc"] = build_nc()
    return _NC_CACHE["nc"]


def kernel(**inputs):
    """Full-input / full-output BigBird block on 8 NeuronCores."""
    from concourse.bass_utils import run_bass_kernel_spmd
    nc = _get_nc()
    in_maps, b2 = host_prep(inputs)
    res = run_bass_kernel_spmd(nc, in_maps, list(range(NCORES)))
    return host_assemble(res.results, b2)

